# revision 1
# baseline (speedup 1.0000x reference)
"""Trainium2 Bass kernel for CIN layer:
    out[b,c,d] = sum_{h,m} W[c, h*M+m] * xk[b,h,d] * x0[b,m,d] + bias[c]

Shapes (hardcoded): x0 [512,40,64] f32, xk [512,128,64] f32,
W [128,5120] f32, b [128] f32 -> out [512,128,64] f32.

Strategy: data-parallel over batch B across 8 cores (64 batches/core).
Per core, columns are the 64*64=4096 (b,d) pairs. The 5120-long (h,m)
contraction is split into 40 chunks of 128 rows with a mixed-radix
partition layout: chunk (g, j) covers m in the 8-wide group g (5 groups)
x h in the 16-wide block j (8 blocks); partition p holds
(m = 8g + p//16, h = 16j + p%16). Then
  outer[p, col] = xkrep_j[p, col] * x0bc_g[p, col]  (DVE TT, bf16 2x)
  psum[q] += w3[g,j][p,c].T @ outer[:, q*512:...]   (PE, accum 40 chunks)
where xkrep_j (xk h-block replicated 8x along partitions) and x0bc_g
(x0 m-group rows replicated 16x) are produced host-side (pure layout,
no arithmetic): only 8 + 5 = 13 replicated tiles total, each reused
across the other loop axis - 3.2x less DMA than a full x0 broadcast.
W is host-gathered to match the chunk layout. Bias-add is fused into
the PSUM->SBUF eviction on ScalarE.
"""

import numpy as np
import ml_dtypes

B, M, H, D, C = 512, 40, 128, 64, 128
N_CORES = 8
BC = B // N_CORES          # 64 batches per core
COLS = BC * D              # 4096 (b,d) columns per core
NG = 8                     # PSUM groups
GW = COLS // NG            # 512 columns per group
MG = 8                     # m-values per chunk group
NMG = M // MG              # 5 m-groups
HB = 128 // MG             # 16 h-values per block
NHB = H // HB              # 8 h-blocks
NCHUNK = NMG * NHB         # 40 contraction chunks

_cache = {}


def _build(reps=1):
    import contextlib

    import concourse.bacc as bacc
    import concourse.mybir as mybir
    from concourse.tile import TileContext

    f32 = mybir.dt.float32
    bf16 = mybir.dt.bfloat16

    nc = bacc.Bacc("TRN2", debug=False, num_devices=N_CORES)

    xkr_d = nc.dram_tensor("xkrep_in", [NHB, 128, COLS], bf16, kind="ExternalInput")
    x0b_d = nc.dram_tensor("x0bc_in", [NMG, 128, COLS], bf16, kind="ExternalInput")
    w3_d = nc.dram_tensor("w3_in", [NCHUNK, 128, C], bf16, kind="ExternalInput")
    bias_d = nc.dram_tensor("bias_in", [C, 1], f32, kind="ExternalInput")
    out_d = nc.dram_tensor("out", [BC, C, D], f32, kind="ExternalOutput")

    with TileContext(nc) as tc:
        with (
            tc.tile_pool(name="const", bufs=1) as cpool,
            tc.tile_pool(name="work", bufs=6) as wpool,
            tc.tile_pool(name="outp", bufs=2) as opool,
            tc.tile_pool(name="psum", bufs=1, space="PSUM") as ppool,
        ):
            # ---- load constants / replicated operand tiles ----
            # Each tile is loaded as two half-column DMAs, all phase-0
            # halves first (in rough first-use order), so phase 0 of the
            # main loop can start after ~half the prologue bytes; Tile's
            # subtile dependency tracking lets the half-column TT reads
            # wait only on their half's DMA.
            HC = COLS // 2
            w3_sb = cpool.tile([128, NCHUNK * C], bf16)
            w3_ap = w3_d.ap().rearrange("k p c -> p k c")

            bias_sb = cpool.tile([128, 1], f32)
            nc.sync.dma_start(out=bias_sb, in_=bias_d.ap())

            xkreps = [None] * NHB
            x0bcs = [None] * NMG
            load_order = [("x", 0), ("0", 0), ("x", 1), ("x", 2), ("0", 1),
                          ("x", 3), ("x", 4), ("0", 2), ("x", 5), ("x", 6),
                          ("0", 3), ("x", 7), ("0", 4)]
            for kind, i in load_order:
                if kind == "x":
                    xkr = cpool.tile(
                        [128, COLS], bf16, name=f"xkr{i}", tag=f"xkr{i}"
                    )
                    xkreps[i] = xkr
                else:
                    x0b = cpool.tile(
                        [128, COLS], bf16, name=f"x0b{i}", tag=f"x0b{i}"
                    )
                    x0bcs[i] = x0b
            nc.sync.dma_start(
                out=w3_sb[:, : NCHUNK * C // 2], in_=w3_ap[:, : NCHUNK // 2, :]
            )
            for ph in range(2):
                for kind, i in load_order:
                    tile_, src = (
                        (xkreps[i], xkr_d.ap()[i])
                        if kind == "x"
                        else (x0bcs[i], x0b_d.ap()[i])
                    )
                    nc.sync.dma_start(
                        out=tile_[:, ph * HC:(ph + 1) * HC],
                        in_=src[:, ph * HC:(ph + 1) * HC],
                    )
                if ph == 0:
                    nc.sync.dma_start(
                        out=w3_sb[:, NCHUNK * C // 2:],
                        in_=w3_ap[:, NCHUNK // 2:, :],
                    )

            loop_ctx = (
                tc.For_i(
                    0, reps, 1,
                    hint_engines=(mybir.EngineType.PE,),
                    staggered_reset=True,
                )
                if reps > 1
                else contextlib.nullcontext()
            )
            with loop_ctx:
                psums = []
                for q in range(NG):
                    ps = ppool.tile([128, GW], f32, name=f"ps{q}", tag=f"ps{q}")
                    psums.append(ps)

                if reps == 1:
                    # Warm the PE's HAM clock-gate (~3.4us of sustained
                    # activity -> 2.4 GHz) with dummy matmuls on scratch
                    # data while the prologue DMAs are still in flight.
                    # Each real first-accumulation MM uses start=True, so
                    # whatever these leave in PSUM is discarded.
                    scratch = cpool.tile([128, GW], bf16)
                    nc.gpsimd.memset(scratch, 0.0)
                    for _ in range(16):
                        nc.tensor.matmul(
                            psums[0],
                            lhsT=scratch[:, :128],
                            rhs=scratch,
                            start=True,
                            stop=True,
                        )

                # ---- main loop: two column phases over the 40 chunks ----
                # Phase ph sweeps all 40 contraction chunks for columns
                # [ph*2048, (ph+1)*2048) into PSUM banks ph*4..ph*4+3, then
                # evicts those banks while the other phase computes - so
                # the eviction + store tail overlaps compute instead of
                # serializing at the end. Within a phase, MMs are issued
                # in groups of GK chunks, bank-major inside the group, so
                # the PE stays on one PSUM bank for GK consecutive
                # matmuls instead of cycling banks every MM (bank cycling
                # measurably degrades PE throughput).
                GK = 5
                NSLOT = GK + 2
                HCOL = COLS // 2
                out_ap = out_d.ap().rearrange("b c d -> c b d")
                bpg = BC // NG  # batches per bank
                for ph in range(2):
                    for k0 in range(0, NCHUNK, GK):
                        outers = []
                        for k in range(k0, k0 + GK):
                            g, j = divmod(k, NHB)
                            outer = wpool.tile(
                                [128, HCOL], bf16, name=f"outer{ph}_{k}",
                                tag=f"outer{k % NSLOT}", bufs=1,
                            )
                            nc.vector.tensor_mul(
                                outer,
                                xkreps[j][:, ph * HCOL:(ph + 1) * HCOL],
                                x0bcs[g][:, ph * HCOL:(ph + 1) * HCOL],
                            )
                            outers.append(outer)
                        for ql in range(NG // 2):
                            qb = ph * (NG // 2) + ql
                            for i, k in enumerate(range(k0, k0 + GK)):
                                nc.tensor.matmul(
                                    psums[qb],
                                    lhsT=w3_sb[:, k * C:(k + 1) * C],
                                    rhs=outers[i][:, ql * GW:(ql + 1) * GW],
                                    start=(k == 0),
                                    stop=(k == NCHUNK - 1),
                                )
                    # bias add + store for this phase's banks
                    for ql in range(NG // 2):
                        qb = ph * (NG // 2) + ql
                        out_sb = opool.tile(
                            [128, GW], f32, name=f"osb{qb}", tag="osb"
                        )
                        nc.scalar.activation(
                            out_sb,
                            psums[qb],
                            mybir.ActivationFunctionType.Identity,
                            bias=bias_sb[:, 0:1],
                            scale=1.0,
                        )
                        nc.sync.dma_start(
                            out=out_ap[:, qb * bpg:(qb + 1) * bpg, :], in_=out_sb
                        )

    nc.compile()
    return nc


def _prep_host(x0, xk, W, b):
    """Host-side layout prep (no arithmetic): shard, transpose, replicate."""
    part = np.arange(128)
    hh = (part % HB)[None, :] + HB * np.arange(NHB)[:, None]   # [NHB, 128]
    mm = (part // HB)[None, :] + MG * np.arange(NMG)[:, None]  # [NMG, 128]

    Wr = W.reshape(C, H, M)
    w3 = np.empty((NCHUNK, 128, C), ml_dtypes.bfloat16)
    for g in range(NMG):
        for j in range(NHB):
            w3[g * NHB + j] = Wr[:, hh[j], mm[g]].T.astype(ml_dtypes.bfloat16)
    bias = np.ascontiguousarray(b.reshape(C, 1)).astype(np.float32)

    in_maps = []
    for k in range(N_CORES):
        x0s = x0[k * BC:(k + 1) * BC]            # [BC, M, D]
        xks = xk[k * BC:(k + 1) * BC]            # [BC, H, D]
        xk2 = (
            np.ascontiguousarray(xks.transpose(1, 0, 2))
            .reshape(H, COLS)
            .astype(ml_dtypes.bfloat16)
        )
        x02 = (
            np.ascontiguousarray(x0s.transpose(1, 0, 2))
            .reshape(M, COLS)
            .astype(ml_dtypes.bfloat16)
        )
        in_maps.append(
            {
                "xkrep_in": np.ascontiguousarray(xk2[hh]),
                "x0bc_in": np.ascontiguousarray(x02[mm]),
                "w3_in": w3,
                "bias_in": bias,
            }
        )
    return in_maps


def _run(in_maps, **kwargs):
    from concourse import bass_utils

    if "nc" not in _cache:
        _cache["nc"] = _build()
    return bass_utils.run_bass_kernel_spmd(
        _cache["nc"], in_maps, core_ids=list(range(N_CORES)), **kwargs
    )


def kernel(x0, xk, W, b, _bench=[None]):
    x0 = np.asarray(x0, dtype=np.float32)
    xk = np.asarray(xk, dtype=np.float32)
    W = np.asarray(W, dtype=np.float32)
    b = np.asarray(b, dtype=np.float32)
    in_maps = _prep_host(x0, xk, W, b)
    res = _run(in_maps)
    _bench[0] = res
    out = np.concatenate([r["out"] for r in res.results], axis=0)
    return out.astype(np.float32, copy=False)



# revision 4
# speedup vs baseline: 1.1266x; 1.1266x over previous
"""Trainium2 Bass kernel for CIN layer:
    out[b,c,d] = sum_{h,m} W[c, h*M+m] * xk[b,h,d] * x0[b,m,d] + bias[c]

Shapes (hardcoded): x0 [512,40,64] f32, xk [512,128,64] f32,
W [128,5120] f32, b [128] f32 -> out [512,128,64] f32.

Strategy: data-parallel over batch B across 8 cores (64 batches/core).
Per core, columns are the 64*64=4096 (b,d) pairs. The 5120-long (h,m)
contraction is split into 40 chunks of 128 rows with a mixed-radix
partition layout: chunk (g, j) covers m in the 8-wide group g (5 groups)
x h in the 16-wide block j (8 blocks); partition p holds
(m = 8g + p//16, h = 16j + p%16). Then
  outer[p, col] = xkrep_j[p, col] * x0bc_g[p, col]  (DVE TT, bf16 2x)
  psum[q] += w3[t][p,c].T @ outer[:, q*512:...]     (PE, accum 40 chunks)
xkrep_j / x0bc_g replicas are produced host-side (pure layout).

The kernel is DVE-bound (the 5120x4096 elementwise outer products at
2 bf16/cycle/lane ~= 85us; PE matmuls are 68us) so everything is
organized to keep the DVE back-to-back: TTs are fused in j-pairs (40
instructions of [128,4096] instead of 80 of [128,2048], halving the
per-instruction SBUF-access overhead), input DMA issue alternates
between the Sync and Activation HWDGE queues so descriptor-gen
serialization (~0.9us per dma_start) never gates the DVE, and tile
halves are loaded in first-use order. Output is written c-major
([C,BC,D]) so each PSUM-bank store is one 2KB descriptor per
partition; the host transposes back. Bias-add is fused into the
PSUM->SBUF eviction (ACT for phase-0 banks mid-kernel; split ACT/DVE
at the tail where the DVE is free).
"""

import numpy as np
import ml_dtypes

B, M, H, D, C = 512, 40, 128, 64, 128
N_CORES = 8
BC = B // N_CORES          # 64 batches per core
COLS = BC * D              # 4096 (b,d) columns per core
NG = 8                     # PSUM banks
GW = COLS // NG            # 512 columns per bank
MG = 8                     # m-values per chunk group
NMG = M // MG              # 5 m-groups
HB = 128 // MG             # 16 h-values per block
NHB = H // HB              # 8 h-blocks
NCHUNK = NMG * NHB         # 40 contraction chunks
HC = COLS // 2             # 2048 columns per phase

# TT issue order: j-pairs outer, g inner. CK[t] = (g, j) of the t-th
# chunk consumed by the PE; w3 is laid out host-side in this order.
CK = [(g, 2 * jp + e) for jp in range(NHB // 2) for g in range(NMG)
      for e in range(2)]

_cache = {}


def _build(reps=1):
    import contextlib

    import concourse.bacc as bacc
    import concourse.mybir as mybir
    from concourse.tile import TileContext

    f32 = mybir.dt.float32
    bf16 = mybir.dt.bfloat16

    nc = bacc.Bacc("TRN2", debug=False, num_devices=N_CORES)

    xkr_d = nc.dram_tensor("xkrep_in", [NHB, 128, COLS], bf16, kind="ExternalInput")
    x0b_d = nc.dram_tensor("x0bc_in", [NMG, 128, COLS], bf16, kind="ExternalInput")
    w3_d = nc.dram_tensor("w3_in", [128, NCHUNK * C], bf16, kind="ExternalInput")
    bias_d = nc.dram_tensor("bias_in", [C, 1], f32, kind="ExternalInput")
    out_d = nc.dram_tensor("out", [C, BC, D], f32, kind="ExternalOutput")

    with TileContext(nc) as tc:
        with (
            tc.tile_pool(name="const", bufs=1) as cpool,
            tc.tile_pool(name="work", bufs=5) as wpool,
            tc.tile_pool(name="outp", bufs=4) as opool,
            tc.tile_pool(name="psum", bufs=1, space="PSUM") as ppool,
        ):
            # ---- persistent input tiles (single big allocations) ----
            xkr_sb = cpool.tile([128, NHB * COLS], bf16)
            x0b_sb = cpool.tile([128, NMG * COLS], bf16)
            w3_sb = cpool.tile([128, NCHUNK * C], bf16)
            bias_sb = cpool.tile([128, 1], f32)

            def xkr(j, ph):
                return xkr_sb[:, j * COLS + ph * HC:j * COLS + (ph + 1) * HC]

            def x0b(g, ph):
                return x0b_sb[:, g * COLS + ph * HC:g * COLS + (ph + 1) * HC]

            # ---- input DMA: first-use order, alternating HWDGE queues.
            # Phase-0 halves of every tile first (phase 0 only touches
            # columns [0, 2048)), then phase-1 halves. w3 goes early on
            # the scalar queue; Tile's subtile tracking lets each TT
            # wait only on the halves it reads.
            nc.sync.dma_start(out=bias_sb, in_=bias_d.ap())
            nc.scalar.dma_start(out=w3_sb[:, :NCHUNK * C // 2],
                                in_=w3_d.ap()[:, :NCHUNK * C // 2])
            tile_order = [("x", 0), ("0", 0), ("x", 1), ("0", 1), ("0", 2),
                          ("0", 3), ("0", 4), ("x", 2), ("x", 3), ("x", 4),
                          ("x", 5), ("x", 6), ("x", 7)]
            engines = [nc.sync, nc.scalar]
            ei = 0
            for ph in range(2):
                for kind, i in tile_order:
                    if kind == "x":
                        dst = xkr(i, ph)
                        src = xkr_d.ap()[i][:, ph * HC:(ph + 1) * HC]
                    else:
                        dst = x0b(i, ph)
                        src = x0b_d.ap()[i][:, ph * HC:(ph + 1) * HC]
                    engines[ei % 2].dma_start(out=dst, in_=src)
                    ei += 1
                if ph == 0:
                    nc.scalar.dma_start(
                        out=w3_sb[:, NCHUNK * C // 2:],
                        in_=w3_d.ap()[:, NCHUNK * C // 2:])

            loop_ctx = (
                tc.For_i(
                    0, reps, 1,
                    hint_engines=(mybir.EngineType.PE,),
                    staggered_reset=True,
                )
                if reps > 1
                else contextlib.nullcontext()
            )
            with loop_ctx:
                psums = []
                for q in range(NG):
                    ps = ppool.tile([128, GW], f32, name=f"ps{q}", tag=f"ps{q}")
                    psums.append(ps)

                if reps == 1:
                    # Warm the PE's HAM clock-gate with small dummy
                    # matmuls on scratch data while the prologue DMAs
                    # are in flight. Real first-accumulation MMs use
                    # start=True, so PSUM garbage is discarded.
                    scratch = cpool.tile([128, 128], bf16)
                    nc.gpsimd.memset(scratch, 0.0)
                    for _ in range(28):
                        nc.tensor.matmul(
                            psums[0][:, :128],
                            lhsT=scratch,
                            rhs=scratch,
                            start=True,
                            stop=True,
                        )

                # ---- main loop: 2 column phases over 20 fused TTs ----
                # Fused TT t covers chunks CK[2t], CK[2t+1] (same g,
                # adjacent j): in0 reads the two xkrep j-blocks via a
                # strided outer dim, in1 reads x0bc_g twice (stride-0
                # outer dim), keeping the DVE in 2x bf16 mode. The PE
                # consumes each fused outer as 2 chunks x 4 banks of
                # 512 columns, accumulating 40 chunks per bank.
                out_ap = out_d.ap()
                bpg = BC // NG  # batches per bank
                for ph in range(2):
                    for t in range(0, NCHUNK, 2):
                        g, j0 = CK[t]
                        outer = wpool.tile(
                            [128, 2 * HC], bf16, name=f"outer{ph}_{t}",
                            tag=f"outer{(t // 2) % 5}", bufs=1,
                        )
                        # two j-blocks: strided outer dim on in0;
                        # stride-0 outer dim on in1 (same g twice)
                        in0 = (xkr_sb[:, j0 * COLS:(j0 + 2) * COLS]
                               .rearrange("p (two c) -> p two c", two=2)
                               [:, :, ph * HC:(ph + 1) * HC])
                        in1 = (x0b(g, ph).unsqueeze(1)
                               .broadcast_to([128, 2, HC]))
                        nc.vector.tensor_tensor(
                            outer.rearrange("p (two c) -> p two c", two=2),
                            in0,
                            in1,
                            mybir.AluOpType.mult,
                        )
                        for ql in range(NG // 2):
                            qb = ph * (NG // 2) + ql
                            for e in range(2):
                                k = t + e
                                nc.tensor.matmul(
                                    psums[qb],
                                    lhsT=w3_sb[:, k * C:(k + 1) * C],
                                    rhs=outer[:, e * HC + ql * GW:
                                              e * HC + (ql + 1) * GW],
                                    start=(k == 0),
                                    stop=(k == NCHUNK - 1),
                                )
                    # bias add + store for this phase's banks.
                    # Phase 0: ACT only (DVE is mid-stream). Phase 1:
                    # split ACT / DVE - the DVE is idle after its last
                    # TT and PSUM reads don't touch its SBUF ports.
                    for ql in range(NG // 2):
                        qb = ph * (NG // 2) + ql
                        out_sb = opool.tile(
                            [128, GW], f32, name=f"osb{qb}", tag="osb"
                        )
                        if ph == 1 and ql % 2 == 1:
                            nc.vector.tensor_scalar_add(
                                out_sb, psums[qb], bias_sb[:, 0:1])
                            nc.sync.dma_start(
                                out=out_ap[:, qb * bpg:(qb + 1) * bpg, :],
                                in_=out_sb)
                        else:
                            nc.scalar.activation(
                                out_sb,
                                psums[qb],
                                mybir.ActivationFunctionType.Identity,
                                bias=bias_sb[:, 0:1],
                                scale=1.0,
                            )
                            nc.scalar.dma_start(
                                out=out_ap[:, qb * bpg:(qb + 1) * bpg, :],
                                in_=out_sb)

    nc.compile()
    return nc


def _prep_host(x0, xk, W, b):
    """Host-side layout prep (no arithmetic): shard, transpose, replicate."""
    part = np.arange(128)
    hh = (part % HB)[None, :] + HB * np.arange(NHB)[:, None]   # [NHB, 128]
    mm = (part // HB)[None, :] + MG * np.arange(NMG)[:, None]  # [NMG, 128]

    Wr = W.reshape(C, H, M)
    # w3[t] = weights for chunk CK[t], laid out [128, NCHUNK*C] so the
    # DMA is contiguous per partition and lhsT slices follow TT order.
    w3 = np.empty((128, NCHUNK * C), ml_dtypes.bfloat16)
    for t, (g, j) in enumerate(CK):
        w3[:, t * C:(t + 1) * C] = Wr[:, hh[j], mm[g]].T.astype(
            ml_dtypes.bfloat16)
    bias = np.ascontiguousarray(b.reshape(C, 1)).astype(np.float32)

    in_maps = []
    for k in range(N_CORES):
        x0s = x0[k * BC:(k + 1) * BC]            # [BC, M, D]
        xks = xk[k * BC:(k + 1) * BC]            # [BC, H, D]
        xk2 = (
            np.ascontiguousarray(xks.transpose(1, 0, 2))
            .reshape(H, COLS)
            .astype(ml_dtypes.bfloat16)
        )
        x02 = (
            np.ascontiguousarray(x0s.transpose(1, 0, 2))
            .reshape(M, COLS)
            .astype(ml_dtypes.bfloat16)
        )
        in_maps.append(
            {
                "xkrep_in": np.ascontiguousarray(xk2[hh]),
                "x0bc_in": np.ascontiguousarray(x02[mm]),
                "w3_in": w3,
                "bias_in": bias,
            }
        )
    return in_maps


def _run(in_maps, **kwargs):
    from concourse import bass_utils

    if "nc" not in _cache:
        _cache["nc"] = _build()
    return bass_utils.run_bass_kernel_spmd(
        _cache["nc"], in_maps, core_ids=list(range(N_CORES)), **kwargs
    )


def kernel(x0, xk, W, b, _bench=[None]):
    x0 = np.asarray(x0, dtype=np.float32)
    xk = np.asarray(xk, dtype=np.float32)
    W = np.asarray(W, dtype=np.float32)
    b = np.asarray(b, dtype=np.float32)
    in_maps = _prep_host(x0, xk, W, b)
    res = _run(in_maps)
    _bench[0] = res
    # per-core out is [C, BC, D]; concatenate batches then put C second
    out = np.concatenate([r["out"] for r in res.results], axis=1)
    return np.ascontiguousarray(out.transpose(1, 0, 2)).astype(
        np.float32, copy=False)
